# revision 18
# baseline (speedup 1.0000x reference)
"""Low-rank layer y = (U^T V) @ x computed as y = U^T @ (V @ x).

Full problem: x [8192, 4096] f32, U/V [8, 8192] f32, y [8192, 4096] f32.
Sharding: batch (columns of x) split across 8 NeuronCores, 512 per core.
Per core the kernel streams its x shard once (16 MiB), reduces it against
V^T to T = V@x [8, 512] in PSUM, then expands with U to y [8192, 512].
Memory-bound: 32 MiB HBM traffic per core.
"""

import numpy as np

L = 8192
RANK = 8
BATCH = 4096
NCORES = 8
BS = BATCH // NCORES  # 512 batch columns per core
P = 128               # SBUF partitions
NCHUNK = L // P       # 64 row-chunks of 128
XG = 16               # L-chunks per input DMA  (16 * 256 KiB = 4 MiB per DMA)
YG = 8                # L-chunks per output DMA

_NC = None  # cached compiled Bass module


def _body(tc, nc, x, vt, u, y, mybir):
    from contextlib import ExitStack

    f32 = mybir.dt.float32
    x3 = x.rearrange("(n p) b -> p n b", p=P)   # [128, 64, 512] view of DRAM
    y3 = y.rearrange("(n p) b -> p n b", p=P)

    # Constraint shaping every choice below: this walrus build encodes at
    # most ONE sync wait per instruction (any engine, incl. DMA). So:
    #  - DMAs never write a reused SBUF slot (x tiles are all distinct);
    #  - each phase's first PE instruction is a dummy matmul absorbing the
    #    weight-tensor DMA wait;
    #  - y stage tiles reuse 2 slots, but their first writer is a copy whose
    #    slot-release dep is redirected onto a tiny same-engine helper op.
    from concourse.tile import add_dep_helper

    NXG = NCHUNK // XG      # 8 input DMA groups, all resident
    NSTAGE = NCHUNK // YG   # 8 output stages, 2 rotating slots
    with ExitStack() as ctx:
        const = ctx.enter_context(tc.tile_pool(name="const", bufs=1))
        xpool = ctx.enter_context(tc.tile_pool(name="xbuf", bufs=NXG))
        tpsum = ctx.enter_context(tc.tile_pool(name="tpsum", bufs=1, space="PSUM"))
        ypsum = ctx.enter_context(tc.tile_pool(name="ypsum", bufs=4, space="PSUM"))
        ystage = ctx.enter_context(tc.tile_pool(name="ystage", bufs=2))

        # Tiny replicated operands.
        vt_sb = const.tile([P, NCHUNK * RANK], f32)   # vt[p, n*8+r] = V[r, n*128+p]
        nc.sync.dma_start(vt_sb[:], vt[:])
        u_sb = const.tile([RANK, L], f32)
        nc.sync.dma_start(u_sb[:], u[:])
        t_sb = const.tile([RANK, BS], f32)
        scr_v = const.tile([1, 16], f32, tag="scr_v")  # DVE helper scratch

        # Dummy matmul reading ONLY vt_sb: absorbs the vt DMA wait so the
        # first real matmul carries a single sync wait.
        warm1 = tpsum.tile([RANK, RANK], f32, tag="warm1")
        nc.tensor.matmul(warm1[:], vt_sb[:, 0:RANK], vt_sb[:, 0:RANK],
                         start=True, stop=True)

        # Phase 1: stream x in XG-chunk groups, accumulate T = V @ x in PSUM.
        t_ps = tpsum.tile([RANK, BS], f32)
        for d in range(NXG):
            xt = xpool.tile([P, XG * BS], f32, tag="xt")
            nc.sync.dma_start(
                xt[:].rearrange("p (n b) -> p n b", b=BS),
                x3[:, d * XG:(d + 1) * XG, :],
            )
            for c in range(XG):
                n = d * XG + c
                nc.tensor.matmul(
                    t_ps[:],
                    vt_sb[:, n * RANK:(n + 1) * RANK],  # lhsT [128, 8]
                    xt[:, c * BS:(c + 1) * BS],         # rhs  [128, 512]
                    start=(n == 0),
                    stop=(n == NCHUNK - 1),
                )
        # On DVE (not ACT) so the ACT engine is entirely unused: every active
        # engine/DMA-lane proc adds a wait to the framework's tail drain,
        # which also has a wait-slot cap.
        nc.vector.tensor_copy(t_sb[:], t_ps[:])

        # Same trick for phase 2: absorb the u DMA wait on a dummy matmul so
        # the first y matmul waits only on the t_sb copy.
        warm2 = tpsum.tile([P, RANK], f32, tag="warm2")
        nc.tensor.matmul(warm2[:], u_sb[:, 0:P], u_sb[:, 0:RANK],
                         start=True, stop=True)

        # Phase 2: y chunk = U_chunk^T @ T, staged through SBUF, YG chunks per
        # DMA. All copies on DVE so same-engine ordering handles stage slot
        # reuse; a zero-input memset helper (sole dep: the store DMA that
        # drained the slot) advances DVE's clock past the slot release.
        dma_outs = []
        for d in range(NSTAGE):
            if d >= 2:
                h = nc.vector.memset(scr_v[0:1, d:d + 1], 0.0)
                add_dep_helper(h.ins, dma_outs[d - 2].ins,
                               reason="stage slot drained by store DMA")
            stage = ystage.tile([P, YG * BS], f32, tag="ys")
            if d >= 2:
                # Zero-input first touch: soaks up the slot-release self-sem
                # so the real copies carry only their PE wait.
                nc.vector.memset(stage[0:1, 0:1], 0.0)
            for c in range(YG):
                n = d * YG + c
                y_ps = ypsum.tile([P, BS], f32, tag="yp")
                nc.tensor.matmul(
                    y_ps[:],
                    u_sb[:, n * P:(n + 1) * P],  # lhsT [8, 128]
                    t_sb[:],                     # rhs  [8, 512]
                    start=True,
                    stop=True,
                )
                nc.vector.tensor_copy(stage[:, c * BS:(c + 1) * BS], y_ps[:])
            # Stores go via SWDGE (gpsimd): fresh DMASW sem lanes, so no
            # HWDGE lane-recycle waits pile onto these instructions.
            dma_outs.append(nc.gpsimd.dma_start(
                y3[:, d * YG:(d + 1) * YG, :],
                stage[:].rearrange("p (n b) -> p n b", b=BS),
            ))


def build_bass():
    import concourse.mybir as mybir
    import concourse.tile as tile
    from concourse import bacc

    # Bacc (not raw Bass): its compile() runs generate_event_semaphores(),
    # which splits multi-sem waits into the 1-wait-per-instruction form the
    # TRN2 ISA requires.
    nc = bacc.Bacc("TRN2", target_bir_lowering=False, debug=False)
    x = nc.dram_tensor("x", [L, BS], mybir.dt.float32, kind="ExternalInput").ap()
    vt = nc.dram_tensor("vt", [P, NCHUNK * RANK], mybir.dt.float32, kind="ExternalInput").ap()
    u = nc.dram_tensor("u", [RANK, L], mybir.dt.float32, kind="ExternalInput").ap()
    y = nc.dram_tensor("y", [L, BS], mybir.dt.float32, kind="ExternalOutput").ap()

    with tile.TileContext(nc) as tc:
        _body(tc, nc, x, vt, u, y, mybir)
    nc.compile()
    return nc


def _get_nc():
    global _NC
    if _NC is None:
        _NC = build_bass()
    return _NC


def make_in_maps(inputs, U, V):
    x = np.asarray(inputs, dtype=np.float32)
    U = np.ascontiguousarray(np.asarray(U, dtype=np.float32))
    V = np.asarray(V, dtype=np.float32)
    # vt[p, n*RANK + r] = V[r, n*128 + p]
    vt = np.ascontiguousarray(
        V.reshape(RANK, NCHUNK, P).transpose(2, 1, 0).reshape(P, NCHUNK * RANK)
    )
    in_maps = []
    for c in range(NCORES):
        xs = np.ascontiguousarray(x[:, c * BS:(c + 1) * BS])
        in_maps.append({"x": xs, "vt": vt, "u": U})
    return in_maps


def kernel(inputs, U, V):
    from concourse import bass_utils

    nc = _get_nc()
    in_maps = make_in_maps(inputs, U, V)
    res = bass_utils.run_bass_kernel_spmd(nc, in_maps, core_ids=list(range(NCORES)))
    return np.concatenate([res.results[c]["y"] for c in range(NCORES)], axis=1)


# revision 20
# speedup vs baseline: 2.9302x; 2.9302x over previous
"""Low-rank layer y = (U^T V) @ x computed as y = U^T @ (V @ x).

Full problem: x [8192, 4096] f32, U/V [8, 8192] f32, y [8192, 4096] f32.
Sharding: batch (columns of x) split across 8 NeuronCores, 512 per core.
Per core the kernel streams its x shard once (16 MiB), reduces it against
V^T to T = V@x [8, 512] in PSUM, then expands with U to y [8192, 512].
Memory-bound: 32 MiB HBM traffic per core.
"""

import numpy as np

L = 8192
RANK = 8
BATCH = 4096
NCORES = 8
BS = BATCH // NCORES  # 512 batch columns per core
P = 128               # SBUF partitions
NCHUNK = L // P       # 64 row-chunks of 128
XG = 16               # L-chunks per input DMA  (16 * 256 KiB = 4 MiB per DMA)
YG = 8                # L-chunks per output DMA

_NC = None  # cached compiled Bass module


def _body(tc, nc, x, vt, u, y, mybir):
    from contextlib import ExitStack

    f32 = mybir.dt.float32
    x3 = x.rearrange("(n p) b -> p n b", p=P)   # [128, 64, 512] view of DRAM
    y3 = y.rearrange("(n p) b -> p n b", p=P)

    # Constraint shaping every choice below: this walrus build encodes at
    # most ONE sync wait per instruction (any engine, incl. DMA). So:
    #  - DMAs never write a reused SBUF slot (x tiles are all distinct);
    #  - each phase's first PE instruction is a dummy matmul absorbing the
    #    weight-tensor DMA wait;
    #  - y stage tiles reuse 2 slots, but their first writer is a copy whose
    #    slot-release dep is redirected onto a tiny same-engine helper op.
    NXG = NCHUNK // XG      # 8 input DMA groups, all resident
    NSTAGE = NCHUNK // YG   # 8 output stages, 2 rotating slots
    with ExitStack() as ctx:
        const = ctx.enter_context(tc.tile_pool(name="const", bufs=1))
        xpool = ctx.enter_context(tc.tile_pool(name="xbuf", bufs=NXG))
        tpsum = ctx.enter_context(tc.tile_pool(name="tpsum", bufs=1, space="PSUM"))
        ypsum = ctx.enter_context(tc.tile_pool(name="ypsum", bufs=4, space="PSUM"))
        ystage = ctx.enter_context(tc.tile_pool(name="ystage", bufs=2))

        # Tiny replicated operands.
        vt_sb = const.tile([P, NCHUNK * RANK], f32)   # vt[p, n*8+r] = V[r, n*128+p]
        nc.sync.dma_start(vt_sb[:], vt[:])
        u_sb = const.tile([RANK, L], f32)
        nc.sync.dma_start(u_sb[:], u[:])
        t_sb = const.tile([RANK, BS], f32)

        # Dummy matmul reading ONLY vt_sb: absorbs the vt DMA wait so the
        # first real matmul carries a single sync wait.
        warm1 = tpsum.tile([RANK, RANK], f32, tag="warm1")
        nc.tensor.matmul(warm1[:], vt_sb[:, 0:RANK], vt_sb[:, 0:RANK],
                         start=True, stop=True)

        # Phase 1: stream x in XG-chunk groups, accumulate T = V @ x in PSUM.
        t_ps = tpsum.tile([RANK, BS], f32)
        for d in range(NXG):
            xt = xpool.tile([P, XG * BS], f32, tag="xt")
            nc.sync.dma_start(
                xt[:].rearrange("p (n b) -> p n b", b=BS),
                x3[:, d * XG:(d + 1) * XG, :],
            )
            for c in range(XG):
                n = d * XG + c
                nc.tensor.matmul(
                    t_ps[:],
                    vt_sb[:, n * RANK:(n + 1) * RANK],  # lhsT [128, 8]
                    xt[:, c * BS:(c + 1) * BS],         # rhs  [128, 512]
                    start=(n == 0),
                    stop=(n == NCHUNK - 1),
                )
        # On DVE (not ACT) so the ACT engine is entirely unused: every active
        # engine/DMA-lane proc adds a wait to the framework's tail drain,
        # which also has a wait-slot cap.
        nc.vector.tensor_copy(t_sb[:], t_ps[:])

        # Same trick for phase 2: absorb the u DMA wait on a dummy matmul so
        # the first y matmul waits only on the t_sb copy.
        warm2 = tpsum.tile([P, RANK], f32, tag="warm2")
        nc.tensor.matmul(warm2[:], u_sb[:, 0:P], u_sb[:, 0:RANK],
                         start=True, stop=True)

        # Phase 2: y chunk = U_chunk^T @ T, staged through SBUF, YG chunks per
        # DMA. All copies on DVE so same-engine ordering handles stage slot
        # reuse; a zero-input memset helper (sole dep: the store DMA that
        # drained the slot) advances DVE's clock past the slot release.
        dma_outs = []
        for d in range(NSTAGE):
            stage = ystage.tile([P, YG * BS], f32, tag="ys")
            for c in range(YG):
                n = d * YG + c
                y_ps = ypsum.tile([P, BS], f32, tag="yp")
                nc.tensor.matmul(
                    y_ps[:],
                    u_sb[:, n * P:(n + 1) * P],  # lhsT [8, 128]
                    t_sb[:],                     # rhs  [8, 512]
                    start=True,
                    stop=True,
                )
                # Alternate PSUM->SBUF copies across ACT and DVE: halves the
                # per-stage copy chain vs a single engine. Multi-sem waits
                # are legal here (Bacc's generate_event_semaphores splits
                # them), so stage-slot reuse needs no helper ops.
                if c % 2 == 0:
                    nc.scalar.copy(stage[:, c * BS:(c + 1) * BS], y_ps[:])
                else:
                    nc.vector.tensor_copy(stage[:, c * BS:(c + 1) * BS], y_ps[:])
            # Stores go via SWDGE (gpsimd): fresh DMASW sem lanes, so no
            # HWDGE lane-recycle waits pile onto these instructions.
            dma_outs.append(nc.gpsimd.dma_start(
                y3[:, d * YG:(d + 1) * YG, :],
                stage[:].rearrange("p (n b) -> p n b", b=BS),
            ))


def build_bass():
    import concourse.mybir as mybir
    import concourse.tile as tile
    from concourse import bacc

    # Bacc (not raw Bass): its compile() runs generate_event_semaphores(),
    # which splits multi-sem waits into the 1-wait-per-instruction form the
    # TRN2 ISA requires.
    nc = bacc.Bacc("TRN2", target_bir_lowering=False, debug=False)
    x = nc.dram_tensor("x", [L, BS], mybir.dt.float32, kind="ExternalInput").ap()
    vt = nc.dram_tensor("vt", [P, NCHUNK * RANK], mybir.dt.float32, kind="ExternalInput").ap()
    u = nc.dram_tensor("u", [RANK, L], mybir.dt.float32, kind="ExternalInput").ap()
    y = nc.dram_tensor("y", [L, BS], mybir.dt.float32, kind="ExternalOutput").ap()

    with tile.TileContext(nc) as tc:
        _body(tc, nc, x, vt, u, y, mybir)
    nc.compile()
    return nc


def _get_nc():
    global _NC
    if _NC is None:
        _NC = build_bass()
    return _NC


def make_in_maps(inputs, U, V):
    x = np.asarray(inputs, dtype=np.float32)
    U = np.ascontiguousarray(np.asarray(U, dtype=np.float32))
    V = np.asarray(V, dtype=np.float32)
    # vt[p, n*RANK + r] = V[r, n*128 + p]
    vt = np.ascontiguousarray(
        V.reshape(RANK, NCHUNK, P).transpose(2, 1, 0).reshape(P, NCHUNK * RANK)
    )
    in_maps = []
    for c in range(NCORES):
        xs = np.ascontiguousarray(x[:, c * BS:(c + 1) * BS])
        in_maps.append({"x": xs, "vt": vt, "u": U})
    return in_maps


def kernel(inputs, U, V):
    from concourse import bass_utils

    nc = _get_nc()
    in_maps = make_in_maps(inputs, U, V)
    res = bass_utils.run_bass_kernel_spmd(nc, in_maps, core_ids=list(range(NCORES)))
    return np.concatenate([res.results[c]["y"] for c in range(NCORES)], axis=1)
